# revision 3
# baseline (speedup 1.0000x reference)
"""GCNConv (dense adjacency, 8192 nodes, 512 feat) on 8 Trainium2 NeuronCores.

Math (matches reference):
    A = adj + I
    deg = A.sum(axis=1); dinv = rsqrt(deg)        (deg >= 1 always)
    h = concat(x[:4096] @ Wr, x[4096:] @ Wd)
    out = leaky_relu(dinv[:,None] * (A @ (dinv[:,None] * h)) + bias, 0.01)

Sharding: rows of A / output row-sharded over 8 cores (1024 rows each).
Host ships, per core, the *transposed* shard adjt = A[rows].T (bf16, with the
self-loop identity folded in on the host) laid out as 64 j-strips of
[128, 1024] so every matmul stationary tile is a contiguous free-dim slice.
h-shards and degree-shards are exchanged on-device with AllGather; everything
else (dinv, feature scaling, SpMM, epilogue) runs on device.
"""

import numpy as np
import ml_dtypes

import concourse.bass as bass
import concourse.tile as tile
from concourse import bacc, mybir
from concourse.bass_utils import run_bass_kernel_spmd

N = 8192
C = 512
NCORES = 8
ROWS = N // NCORES       # 1024 rows per core
P = 128
KT = N // P              # 64 contraction tiles
MT = ROWS // P           # 8 output row tiles per core
FT = C // P              # 4 feature tiles for x @ W

F32 = mybir.dt.float32
BF16 = mybir.dt.bfloat16


def _emit(nc, tc, dram, io, r):
    """Emit one full GCN pass. `r` tags pools/tiles for program repetition."""
    adjt_d, xt_d, w_d, biasb_d, out_d = io

    h_bounce = dram.tile([ROWS, C], F32, name=f"h_bounce{r}")
    h_full = dram.tile([N, C], F32, addr_space="Shared", name=f"h_full{r}")
    deg_b = dram.tile([1, ROWS], F32, name=f"deg_b{r}")
    deg_all = dram.tile([NCORES, ROWS], F32, addr_space="Shared",
                        name=f"deg_all{r}")

    with tc.tile_pool(name=f"const{r}", bufs=1) as const_pool, \
         tc.tile_pool(name=f"misc{r}", bufs=1) as misc_pool:
        ones_t = const_pool.tile([P, 1], BF16)
        nc.gpsimd.memset(ones_t[:], 1.0)
        bias_t = const_pool.tile([P, C], F32)
        nc.sync.dma_start(bias_t[:], biasb_d.ap())

        # ---------------- Phase 1: h_shard = x_shard @ W ----------
        with tc.tile_pool(name=f"xw{r}", bufs=1) as xw_pool, \
             tc.tile_pool(name=f"hps{r}", bufs=2, space="PSUM") as hps_pool, \
             tc.tile_pool(name=f"hsb{r}", bufs=2) as hsb_pool, \
             tc.tile_pool(name=f"degps{r}", bufs=1, space="PSUM") as degps_pool, \
             tc.tile_pool(name=f"adjs1{r}", bufs=4) as adjs1_pool:
            xt_tiles = []
            w_tiles = []
            for ft in range(FT):
                xt_t = xw_pool.tile([P, ROWS], F32, tag=f"xt{ft}")
                nc.sync.dma_start(xt_t[:], xt_d.ap()[ft * P:(ft + 1) * P, :])
                xt_tiles.append(xt_t)
                w_t = xw_pool.tile([P, C], F32, tag=f"w{ft}")
                nc.sync.dma_start(w_t[:], w_d.ap()[ft * P:(ft + 1) * P, :])
                w_tiles.append(w_t)

            for mt in range(MT):
                h_ps = hps_pool.tile([P, C], F32)
                for ft in range(FT):
                    nc.tensor.matmul(
                        h_ps[:],
                        lhsT=xt_tiles[ft][:, mt * P:(mt + 1) * P],
                        rhs=w_tiles[ft][:],
                        start=(ft == 0), stop=(ft == FT - 1))
                h_sb = hsb_pool.tile([P, C], F32)
                nc.scalar.copy(h_sb[:], h_ps[:])
                nc.sync.dma_start(h_bounce[mt * P:(mt + 1) * P, :], h_sb[:])

            nc.gpsimd.collective_compute(
                "AllGather", mybir.AluOpType.bypass,
                replica_groups=[list(range(NCORES))],
                ins=[h_bounce.opt()], outs=[h_full.opt()])

            # ------------ Phase 2: deg = row sums of A shard -------
            deg_ps = [degps_pool.tile([1, C], F32, tag=f"degp{i}",
                                      name=f"degp{i}")
                      for i in range(2)]
            for jt in range(KT):
                strip = adjs1_pool.tile([P, ROWS], BF16, tag="strip")
                nc.sync.dma_start(strip[:], adjt_d.ap()[jt])
                for half in range(2):
                    nc.tensor.matmul(
                        deg_ps[half][:],
                        lhsT=ones_t[:],
                        rhs=strip[:, half * C:(half + 1) * C],
                        start=(jt == 0), stop=(jt == KT - 1))
            deg_sb = misc_pool.tile([1, ROWS], F32, tag="degsb")
            for half in range(2):
                nc.vector.tensor_copy(
                    deg_sb[:, half * C:(half + 1) * C], deg_ps[half][:])
            nc.sync.dma_start(deg_b[:], deg_sb[:])

            nc.gpsimd.collective_compute(
                "AllGather", mybir.AluOpType.bypass,
                replica_groups=[list(range(NCORES))],
                ins=[deg_b.opt()], outs=[deg_all.opt()])

        # ---------------- Phase 3: dinv ---------------------------
        # deg_all flat index = kt*128 + p  ->  SBUF [p, kt]
        degk_sb = misc_pool.tile([P, KT], F32, tag="degk")
        nc.sync.dma_start(
            degk_sb[:], deg_all.rearrange("c (m p) -> p (c m)", p=P))
        rink = misc_pool.tile([P, KT], F32, tag="rink")
        nc.vector.reciprocal(rink[:], degk_sb[:])
        dinv_sb = misc_pool.tile([P, KT], F32, tag="dinv")
        nc.scalar.sqrt(dinv_sb[:], rink[:])

        degm_sb = misc_pool.tile([P, MT], F32, tag="degm")
        nc.sync.dma_start(
            degm_sb[:], deg_b.rearrange("o (m p) -> (o p) m", p=P))
        rinm = misc_pool.tile([P, MT], F32, tag="rinm")
        nc.vector.reciprocal(rinm[:], degm_sb[:])
        dinvr_sb = misc_pool.tile([P, MT], F32, tag="dinvr")
        nc.scalar.sqrt(dinvr_sb[:], rinm[:])

        # ---------------- Phase 4+5+6 -----------------------------
        with tc.tile_pool(name=f"hstream{r}", bufs=3) as hstream_pool, \
             tc.tile_pool(name=f"gpool{r}", bufs=1) as g_pool, \
             tc.tile_pool(name=f"adjs2{r}", bufs=4) as adjs2_pool, \
             tc.tile_pool(name=f"mmps{r}", bufs=1, space="PSUM") as mmps_pool, \
             tc.tile_pool(name=f"ep{r}", bufs=3) as ep_pool:
            # g tiles: g[kt] = dinv[kt-slice] * h_full[kt-slice]  (bf16)
            g_tiles = []
            for kt in range(KT):
                h_t = hstream_pool.tile([P, C], F32, tag="hst")
                nc.sync.dma_start(h_t[:], h_full[kt * P:(kt + 1) * P, :])
                g_t = g_pool.tile([P, C], BF16, tag=f"g{kt}", name=f"g{kt}")
                nc.vector.tensor_scalar_mul(
                    g_t[:], h_t[:], dinv_sb[:, kt:kt + 1])
                g_tiles.append(g_t)

            mm_ps = [mmps_pool.tile([P, C], F32, tag=f"mm{mt}", name=f"mm{mt}")
                     for mt in range(MT)]
            # main matmul: all 8 m-tiles accumulate across the k loop
            for kt in range(KT - 1):
                strip2 = adjs2_pool.tile([P, ROWS], BF16, tag="strip2")
                nc.sync.dma_start(strip2[:], adjt_d.ap()[kt])
                for mt in range(MT):
                    nc.tensor.matmul(
                        mm_ps[mt][:],
                        lhsT=strip2[:, mt * P:(mt + 1) * P],
                        rhs=g_tiles[kt][:],
                        start=(kt == 0), stop=False)
            # last k-tile interleaved with the epilogue so the PE and
            # ACT/DVE pipeline instead of a serial epilogue tail
            stripL = adjs2_pool.tile([P, ROWS], BF16, tag="strip2")
            nc.sync.dma_start(stripL[:], adjt_d.ap()[KT - 1])
            for mt in range(MT):
                nc.tensor.matmul(
                    mm_ps[mt][:],
                    lhsT=stripL[:, mt * P:(mt + 1) * P],
                    rhs=g_tiles[KT - 1][:],
                    start=False, stop=True)
                t1 = ep_pool.tile([P, C], F32, tag="t1")
                nc.scalar.mul(t1[:], mm_ps[mt][:], dinvr_sb[:, mt:mt + 1])
                t2 = ep_pool.tile([P, C], F32, tag="t2")
                nc.vector.tensor_add(t2[:], t1[:], bias_t[:])
                t3 = ep_pool.tile([P, C], F32, tag="t3")
                nc.scalar.activation(
                    t3[:], t2[:], mybir.ActivationFunctionType.Lrelu,
                    alpha=0.01)
                nc.sync.dma_start(out_d.ap()[mt * P:(mt + 1) * P, :], t3[:])


def build_kernel(reps: int = 1):
    """Build and compile the SPMD Bass program (identical on all 8 cores).

    reps > 1 repeats the whole pipeline inside one NEFF (timing only)."""
    nc = bacc.Bacc("TRN2", target_bir_lowering=False, debug=False,
                   num_devices=NCORES)

    adjt_d = nc.dram_tensor("adjt", [KT, P, ROWS], BF16, kind="ExternalInput")
    xt_d = nc.dram_tensor("xt", [C, ROWS], F32, kind="ExternalInput")
    w_d = nc.dram_tensor("w", [C, C], F32, kind="ExternalInput")
    biasb_d = nc.dram_tensor("biasb", [P, C], F32, kind="ExternalInput")
    out_d = nc.dram_tensor("out", [ROWS, C], F32, kind="ExternalOutput")
    io = (adjt_d, xt_d, w_d, biasb_d, out_d)

    with tile.TileContext(nc) as tc:
        with tc.tile_pool(name="dram", bufs=1, space="DRAM") as dram:
            for r in range(reps):
                _emit(nc, tc, dram, io, r)

    nc.compile()
    return nc


def prepare_inputs(x, adj, weightr, weightd, bias):
    """Host-side sharding/layout. Returns in_maps for the 8 cores."""
    x = np.asarray(x, dtype=np.float32)
    adj = np.asarray(adj, dtype=np.float32)
    weightr = np.ascontiguousarray(np.asarray(weightr, dtype=np.float32))
    weightd = np.ascontiguousarray(np.asarray(weightd, dtype=np.float32))
    bias = np.asarray(bias, dtype=np.float32)

    biasb = np.ascontiguousarray(np.broadcast_to(bias[None, :], (P, C)))
    idx = np.arange(ROWS)

    in_maps = []
    for c in range(NCORES):
        rows = slice(c * ROWS, (c + 1) * ROWS)
        # adjt = A[rows].T  with A = adj + I (self-loop folded in on host)
        at = adj[rows, :].T.astype(ml_dtypes.bfloat16)   # [N, ROWS] contiguous
        at[c * ROWS + idx, idx] += 1                      # 0/1 -> exact in bf16
        adjt = at.reshape(KT, P, ROWS)
        xt = np.ascontiguousarray(x[rows, :].T)           # [C, ROWS]
        w = weightr if c < NCORES // 2 else weightd
        in_maps.append({"adjt": adjt, "xt": xt, "w": w, "biasb": biasb})
    return in_maps


_NC_CACHE = {}


def kernel(x, adj, weightr, weightd, bias):
    if "nc" not in _NC_CACHE:
        _NC_CACHE["nc"] = build_kernel(reps=1)
    nc = _NC_CACHE["nc"]
    in_maps = prepare_inputs(x, adj, weightr, weightd, bias)
    res = run_bass_kernel_spmd(nc, in_maps, list(range(NCORES)))
    out = np.concatenate([res.results[c]["out"] for c in range(NCORES)], axis=0)
    return out


# revision 9
# speedup vs baseline: 1.1776x; 1.1776x over previous
"""GCNConv (dense adjacency, 8192 nodes, 512 feat) on 8 Trainium2 NeuronCores.

Math (matches reference):
    A = adj + I
    deg = A.sum(axis=1); dinv = rsqrt(deg)        (deg >= 1 always)
    h = concat(x[:4096] @ Wr, x[4096:] @ Wd)
    out = leaky_relu(dinv[:,None] * (A @ (dinv[:,None] * h)) + bias, 0.01)

Sharding: rows of A / output row-sharded over 8 cores (1024 rows each).
Host ships, per core, the *transposed* shard adjt = A[rows].T with the
self-loop identity folded in:
  - bf16 j-strips [64, 128, 1024]: moving operand of the main matmul
  - fp8e4 DoubleRow packs [32, 128, 2048] (two j-rows per partition):
    degree pass at 2 elem/lane/cycle; 0/1/2 are exact in fp8 and the
    accumulate is fp32 PSUM, so deg is exact.
h-shards and degree-shards are exchanged on-device with AllGather.

Main matmul is computed transposed (out.T = g.T @ A.T): the stationary
operand is a [128,128] feature-chunk of g, the moving operand is a [128,512]
half of an adjt strip — half the LDWEIGHTS of the natural orientation, and
the epilogue's bias becomes per-partition so it fuses into the LeakyReLU
activation. The per-row dinv scaling becomes a free-axis multiply against a
broadcast tile built on-device with a K=1 matmul.

DMA ops are batched (4 strips / 2MB h chunks per descriptor-gen op) because
the hardware descriptor generator costs ~625ns per op, serialized.
"""

import numpy as np
import ml_dtypes

import concourse.bass as bass
import concourse.tile as tile
from concourse import bacc, mybir
from concourse.bass_utils import run_bass_kernel_spmd

N = 8192
C = 512
NCORES = 8
ROWS = N // NCORES       # 1024 rows per core
P = 128
KT = N // P              # 64 contraction tiles
MT = ROWS // P           # 8 output row tiles per core
FT = C // P              # 4 feature tiles for x @ W
CC = C // P              # 4 feature chunks (stationary side of main matmul)
SPK = 4                  # bf16 j-strips per DMA pack
NPK = KT // SPK          # 16 bf16 packs
NTAILPK = 2              # trailing packs whose epilogues overlap (8 strips)
NDR = KT // 2            # 32 fp8 DoubleRow packs (256 j-rows each)

F32 = mybir.dt.float32
BF16 = mybir.dt.bfloat16
FP8 = mybir.dt.float8e4


def _emit(nc, tc, dram, io, r, sim_mode=False):
    """Emit one full GCN pass. `r` tags pools/tiles for program repetition.

    sim_mode replaces collectives with local DMA stand-ins so the program
    can run under the single-core TimelineSim cost model."""
    adjt_d, adjt8_d, xt_d, w_d, biasc_d, out_d = io

    h_bounce = dram.tile([ROWS, C], BF16, name=f"h_bounce{r}")
    h_full = dram.tile([N, C], BF16, addr_space="Shared", name=f"h_full{r}")
    deg_b = dram.tile([1, ROWS], F32, name=f"deg_b{r}")
    deg_all = dram.tile([NCORES, ROWS], F32, addr_space="Shared",
                        name=f"deg_all{r}")

    with tc.tile_pool(name=f"const{r}", bufs=1) as const_pool, \
         tc.tile_pool(name=f"misc{r}", bufs=1) as misc_pool, \
         tc.tile_pool(name=f"hbig{r}", bufs=1) as hbig_pool:
        ones8_t = const_pool.tile([P, 2, 16], FP8)
        nc.gpsimd.memset(ones8_t[:], 1.0)
        ones1_t = const_pool.tile([1, P], F32)
        nc.gpsimd.memset(ones1_t[:], 1.0)
        bias_pp = const_pool.tile([P, CC], F32)
        nc.sync.dma_start(bias_pp[:],
                          biasc_d.ap().rearrange("(cc p) -> p cc", p=P))
        dinvr_bc = const_pool.tile([P, ROWS], F32)
        # h (later scaled in place into g): one resident [128, 64, 512] tile
        hg_t = hbig_pool.tile([P, KT, C], BF16)

        # ---------------- Phase 1: h_shard = x_shard @ W (bf16) ----
        with tc.tile_pool(name=f"xw{r}", bufs=1) as xw_pool, \
             tc.tile_pool(name=f"hps{r}", bufs=2, space="PSUM") as hps_pool, \
             tc.tile_pool(name=f"hsb{r}", bufs=2) as hsb_pool, \
             tc.tile_pool(name=f"degps{r}", bufs=1, space="PSUM") as degps_pool, \
             tc.tile_pool(name=f"bcps{r}", bufs=1, space="PSUM") as bcps_pool, \
             tc.tile_pool(name=f"adjs1{r}", bufs=4) as adjs1_pool:
            xt_t = xw_pool.tile([P, FT, ROWS], BF16)
            nc.sync.dma_start(
                xt_t[:], xt_d.ap().rearrange("(f p) i -> p f i", p=P))
            w_t = xw_pool.tile([P, FT, C], BF16)
            nc.sync.dma_start(
                w_t[:], w_d.ap().rearrange("(f p) c -> p f c", p=P))

            for mt in range(MT):
                h_ps = hps_pool.tile([P, C], F32)
                for ft in range(FT):
                    nc.tensor.matmul(
                        h_ps[:],
                        lhsT=xt_t[:, ft, mt * P:(mt + 1) * P],
                        rhs=w_t[:, ft, :],
                        start=(ft == 0), stop=(ft == FT - 1))
                h_sb = hsb_pool.tile([P, C], BF16)
                nc.scalar.copy(h_sb[:], h_ps[:])
                nc.sync.dma_start(h_bounce[mt * P:(mt + 1) * P, :], h_sb[:])

            if sim_mode:
                nc.sync.dma_start(h_full[0:ROWS, :], h_bounce[:])
            else:
                nc.gpsimd.collective_compute(
                    "AllGather", mybir.AluOpType.bypass,
                    replica_groups=[list(range(NCORES))],
                    ins=[h_bounce.opt()], outs=[h_full.opt()])

            # h_full -> SBUF in 4 chunks of 2MB (becomes g in phase 4)
            for q in range(4):
                nc.sync.dma_start(
                    hg_t[:, q * (KT // 4):(q + 1) * (KT // 4), :],
                    h_full[q * (N // 4):(q + 1) * (N // 4), :].rearrange(
                        "(k p) c -> p k c", p=P))

            # ------------ Phase 2: deg = row sums of A shard -------
            # fp8 DoubleRow packs: two j-rows per partition, ones weights
            deg_ps = [degps_pool.tile([1, C], F32, tag=f"degp{i}",
                                      name=f"degp{i}")
                      for i in range(2)]
            for dq in range(NDR // 2):      # DMA two DR packs at once
                pk8 = adjs1_pool.tile([P, 2, 2048], FP8, tag="pk8")
                nc.sync.dma_start(
                    pk8[:], adjt8_d.ap()[2 * dq:2 * dq + 2].rearrange(
                        "s p i -> p s i"))
                for s in range(2):
                    q = 2 * dq + s
                    r3 = pk8[:, s, :].rearrange(
                        "p (two i) -> p two i", two=2)
                    for half in range(2):
                        nc.tensor.matmul(
                            deg_ps[half][:],
                            lhsT=ones8_t[:, :, 0:1],
                            rhs=r3[:, :, half * C:(half + 1) * C],
                            perf_mode=mybir.MatmulPerfMode.DoubleRow,
                            start=(q == 0), stop=(q == NDR - 1))
            deg_sb = misc_pool.tile([1, ROWS], F32, tag="degsb")
            for half in range(2):
                nc.vector.tensor_copy(
                    deg_sb[:, half * C:(half + 1) * C], deg_ps[half][:])
            nc.sync.dma_start(deg_b[:], deg_sb[:])

            if sim_mode:
                nc.sync.dma_start(deg_all[0:1, :], deg_b[:])
            else:
                nc.gpsimd.collective_compute(
                    "AllGather", mybir.AluOpType.bypass,
                    replica_groups=[list(range(NCORES))],
                    ins=[deg_b.opt()], outs=[deg_all.opt()])

            # dinv for the core's own rows, broadcast across partitions
            # (no AllGather dependency; overlaps the deg AllGather)
            rrow = misc_pool.tile([1, ROWS], F32, tag="rrow")
            nc.vector.reciprocal(rrow[:], deg_sb[:])
            drow = misc_pool.tile([1, ROWS], F32, tag="drow")
            nc.scalar.sqrt(drow[:], rrow[:])
            bc_ps = bcps_pool.tile([P, ROWS], F32)
            for half in range(2):
                nc.tensor.matmul(
                    bc_ps[:, half * C:(half + 1) * C],
                    lhsT=ones1_t[:],
                    rhs=drow[:, half * C:(half + 1) * C],
                    start=True, stop=True)
            nc.vector.tensor_copy(dinvr_bc[:], bc_ps[:])

        # ---------------- Phase 3: dinv (all nodes) ---------------
        # deg_all flat index = kt*128 + p  ->  SBUF [p, kt]
        degk_sb = misc_pool.tile([P, KT], F32, tag="degk")
        nc.sync.dma_start(
            degk_sb[:], deg_all.rearrange("c (m p) -> p (c m)", p=P))
        rink = misc_pool.tile([P, KT], F32, tag="rink")
        nc.vector.reciprocal(rink[:], degk_sb[:])
        dinv_sb = misc_pool.tile([P, KT], F32, tag="dinv")
        nc.scalar.sqrt(dinv_sb[:], rink[:])

        # ---------------- Phase 4+5+6 -----------------------------
        with tc.tile_pool(name=f"adjs2{r}", bufs=5) as adjs2_pool, \
             tc.tile_pool(name=f"mmps{r}", bufs=1, space="PSUM") as mmps_pool, \
             tc.tile_pool(name=f"ep{r}", bufs=4) as ep_pool:
            # Phase 4: g = dinv * h, in place on the resident h tile
            for kt in range(KT):
                sl = hg_t[:, kt, :]
                nc.vector.tensor_scalar_mul(sl, sl, dinv_sb[:, kt:kt + 1])

            # out.T accumulators: one [128, 1024] (2 PSUM banks) per chunk
            mm_ps = [mmps_pool.tile([P, ROWS], F32, tag=f"mm{cc}",
                                    name=f"mm{cc}")
                     for cc in range(CC)]

            def mm(cc, kt, strip_ap, start, stop):
                for half in range(2):
                    nc.tensor.matmul(
                        mm_ps[cc][:, half * C:(half + 1) * C],
                        lhsT=hg_t[:, kt, cc * P:(cc + 1) * P],
                        rhs=strip_ap[:, half * C:(half + 1) * C],
                        start=start, stop=stop)

            def load_pack(pk):
                t = adjs2_pool.tile([P, SPK, ROWS], BF16, tag="pk2",
                                    name=f"pk2_{pk}")
                nc.sync.dma_start(
                    t[:], adjt_d.ap()[SPK * pk:SPK * (pk + 1)].rearrange(
                        "s p i -> p s i"))
                return t

            for pk in range(NPK - NTAILPK):
                t = load_pack(pk)
                for s in range(SPK):
                    kt = SPK * pk + s
                    for cc in range(CC):
                        mm(cc, kt, t[:, s, :],
                           start=(kt == 0), stop=False)

            # trailing packs: per-cc bursts so each chunk's epilogue
            # overlaps the next chunk's matmuls on the PE
            tail_tiles = [load_pack(pk)
                          for pk in range(NPK - NTAILPK, NPK)]
            ktail0 = SPK * (NPK - NTAILPK)
            for cc in range(CC):
                for kt in range(ktail0, KT):
                    ti = (kt - ktail0) // SPK
                    s = (kt - ktail0) % SPK
                    mm(cc, kt, tail_tiles[ti][:, s, :],
                       start=False, stop=(kt == KT - 1))
                t1 = ep_pool.tile([P, ROWS], F32, tag="t1")
                nc.vector.tensor_mul(t1[:], mm_ps[cc][:], dinvr_bc[:])
                t2 = ep_pool.tile([P, ROWS], F32, tag="t2")
                nc.scalar.activation(
                    t2[:], t1[:], mybir.ActivationFunctionType.Lrelu,
                    bias=bias_pp[:, cc:cc + 1], alpha=0.01)
                nc.sync.dma_start(out_d.ap()[cc * P:(cc + 1) * P, :], t2[:])


def build_kernel(reps: int = 1, sim_mode: bool = False):
    """Build and compile the SPMD Bass program (identical on all 8 cores).

    reps > 1 repeats the whole pipeline inside one NEFF (timing only)."""
    nc = bacc.Bacc("TRN2", target_bir_lowering=False, debug=False,
                   num_devices=NCORES)

    adjt_d = nc.dram_tensor("adjt", [KT, P, ROWS], BF16, kind="ExternalInput")
    adjt8_d = nc.dram_tensor("adjt8", [NDR, P, 2048], FP8, kind="ExternalInput")
    xt_d = nc.dram_tensor("xt", [C, ROWS], BF16, kind="ExternalInput")
    w_d = nc.dram_tensor("w", [C, C], BF16, kind="ExternalInput")
    biasc_d = nc.dram_tensor("biasc", [C], F32, kind="ExternalInput")
    out_d = nc.dram_tensor("out", [C, ROWS], F32, kind="ExternalOutput")
    io = (adjt_d, adjt8_d, xt_d, w_d, biasc_d, out_d)

    with tile.TileContext(nc) as tc:
        with tc.tile_pool(name="dram", bufs=1, space="DRAM") as dram:
            for r in range(reps):
                _emit(nc, tc, dram, io, r, sim_mode=sim_mode)

    nc.compile()
    return nc


def prepare_inputs(x, adj, weightr, weightd, bias):
    """Host-side sharding/layout. Returns in_maps for the 8 cores."""
    x = np.asarray(x, dtype=np.float32)
    adj = np.asarray(adj, dtype=np.float32)
    weightr = np.asarray(weightr, dtype=np.float32)
    weightd = np.asarray(weightd, dtype=np.float32)
    bias = np.ascontiguousarray(np.asarray(bias, dtype=np.float32))

    wr16 = weightr.astype(ml_dtypes.bfloat16)
    wd16 = weightd.astype(ml_dtypes.bfloat16)
    idx = np.arange(ROWS)
    # A values are only 0/1/2: build uint8 once, then LUT-cast (fast + exact)
    lut16 = np.array([0x0000, 0x3F80, 0x4000], dtype=np.uint16)  # bf16 bits
    lut8 = np.array([0x00, 0x38, 0x40], dtype=np.uint8)          # e4m3 bits

    in_maps = []
    for c in range(NCORES):
        rows = slice(c * ROWS, (c + 1) * ROWS)
        ai = adj[rows, :].T.astype(np.uint8)             # [N, ROWS] 0/1
        ai[c * ROWS + idx, idx] += 1                     # fold in self-loop
        adjt = lut16[ai].view(ml_dtypes.bfloat16).reshape(KT, P, ROWS)
        # DoubleRow packs: [32, 128, 2048], row p = [j=q*256+p | j=q*256+128+p]
        adjt8 = np.ascontiguousarray(
            lut8[ai].view(ml_dtypes.float8_e4m3)
            .reshape(NDR, 2, P, ROWS).transpose(0, 2, 1, 3)
        ).reshape(NDR, P, 2048)
        xt = np.ascontiguousarray(x[rows, :].T).astype(ml_dtypes.bfloat16)
        w = wr16 if c < NCORES // 2 else wd16
        in_maps.append({"adjt": adjt, "adjt8": adjt8, "xt": xt, "w": w,
                        "biasc": bias})
    return in_maps


_NC_CACHE = {}


def kernel(x, adj, weightr, weightd, bias):
    if "nc" not in _NC_CACHE:
        _NC_CACHE["nc"] = build_kernel(reps=1)
    nc = _NC_CACHE["nc"]
    in_maps = prepare_inputs(x, adj, weightr, weightd, bias)
    res = run_bass_kernel_spmd(nc, in_maps, list(range(NCORES)))
    out = np.concatenate(
        [np.ascontiguousarray(res.results[c]["out"].T) for c in range(NCORES)],
        axis=0)
    return out


# revision 15
# speedup vs baseline: 1.6053x; 1.3632x over previous
"""GCNConv (dense adjacency, 8192 nodes, 512 feat) on 8 Trainium2 NeuronCores.

Math (matches reference):
    A = adj + I
    deg = A.sum(axis=1); dinv = rsqrt(deg)        (deg >= 1 always)
    h = concat(x[:4096] @ Wr, x[4096:] @ Wd)
    out = leaky_relu(dinv[:,None] * (A @ (dinv[:,None] * h)) + bias, 0.01)

Sharding: rows of A / output row-sharded over 8 cores (1024 rows each).
Host ships, per core, the *transposed* shard adjt = A[rows].T with the
self-loop identity folded in:
  - bf16 j-strips [64, 128, 1024]: moving operand of the main matmul
  - fp8e4 DoubleRow packs [32, 128, 2048] (two j-rows per partition):
    degree pass at 2 elem/lane/cycle; 0/1/2 are exact in fp8 and the
    accumulate is fp32 PSUM, so deg is exact.
h-shards and degree-shards are exchanged on-device with AllGather.

Main matmul is computed transposed (out.T = g.T @ A.T): the stationary
operand is a [128,128] feature-chunk of g, the moving operand is a [128,512]
half of an adjt strip — half the LDWEIGHTS of the natural orientation, and
the epilogue's bias becomes per-partition so it fuses into the LeakyReLU
activation. The per-row dinv scaling becomes a free-axis multiply against a
broadcast tile built on-device with a K=1 matmul.

DMA ops are batched (4 strips / 2MB h chunks per descriptor-gen op) because
the hardware descriptor generator costs ~625ns per op, serialized.
"""

import numpy as np
import ml_dtypes

import concourse.bass as bass
import concourse.tile as tile
from concourse.masks import make_identity
from concourse import bacc, mybir
from concourse.bass_utils import run_bass_kernel_spmd

N = 8192
C = 512
NCORES = 8
ROWS = N // NCORES       # 1024 rows per core
P = 128
KT = N // P              # 64 contraction tiles
MT = ROWS // P           # 8 output row tiles per core
FT = C // P              # 4 feature tiles for x @ W
CC = C // P              # 4 feature chunks (stationary side of main matmul)
SPK = 4                  # bf16 j-strips per DMA pack
NPK = KT // SPK          # 16 bf16 packs
NTAILPK = 2              # trailing packs whose epilogues overlap (8 strips)
NDR = KT // 2            # 32 fp8 DoubleRow packs (256 j-rows each)

F32 = mybir.dt.float32
BF16 = mybir.dt.bfloat16
FP8 = mybir.dt.float8e4


def _emit(nc, tc, dram, io, r, sim_mode=False, parts="all"):
    """Emit one full GCN pass. `r` tags pools/tiles for program repetition.

    sim_mode replaces collectives with local DMA stand-ins so the program
    can run under the single-core TimelineSim cost model."""
    adjt_d, adjt8_d, xt_d, w_d, biasc_d, out_d = io

    h_bounce = dram.tile([ROWS, C], BF16, name=f"h_bounce{r}")
    h_full = dram.tile([N, C], BF16, addr_space="Shared", name=f"h_full{r}")
    deg_b = dram.tile([1, ROWS], F32, name=f"deg_b{r}")
    deg_all = dram.tile([NCORES, ROWS], F32, addr_space="Shared",
                        name=f"deg_all{r}")

    with tc.tile_pool(name=f"const{r}", bufs=1) as const_pool, \
         tc.tile_pool(name=f"misc{r}", bufs=1) as misc_pool, \
         tc.tile_pool(name=f"hbig{r}", bufs=1) as hbig_pool:
        ones8_t = const_pool.tile([P, 2, 16], FP8)
        nc.gpsimd.memset(ones8_t[:], 1.0)
        ones1_t = const_pool.tile([1, P], F32)
        nc.gpsimd.memset(ones1_t[:], 1.0)
        bias_pp = const_pool.tile([P, CC], F32)
        nc.sync.dma_start(bias_pp[:],
                          biasc_d.ap().rearrange("(cc p) -> p cc", p=P))
        dinvr_bc = const_pool.tile([P, ROWS], F32)
        # h (later scaled in place into g): one resident [128, 64, 512] tile
        hg_t = hbig_pool.tile([P, KT, C], BF16)

        if parts == "mm":
            # timing isolation: skip h/deg/AG; fill hg with adjacency bytes
            # (benign 0/1/2 values) and use unit scale factors
            for q in range(4):
                dst = hg_t[:, q * (KT // 4):(q + 1) * (KT // 4), :].rearrange(
                    "p k c -> p (k c)").rearrange("p (s i) -> p s i", s=8)
                nc.sync.dma_start(
                    dst, adjt_d.ap()[8 * q:8 * q + 8].rearrange("s p i -> p s i"))
            nc.vector.memset(dinvr_bc[:], 1.0)
            dinv_sb = misc_pool.tile([P, KT], F32, tag="dinv")
            nc.vector.memset(dinv_sb[:], 1.0)
            _mm_tail(nc, tc, r, hg_t, dinv_sb, dinvr_bc, bias_pp,
                     (None, None, None, None, None, out_d), io[0])
            return

        # ---------------- Phase 1: h_shard = x_shard @ W (bf16) ----
        with tc.tile_pool(name=f"xw{r}", bufs=1) as xw_pool, \
             tc.tile_pool(name=f"hps{r}", bufs=2, space="PSUM") as hps_pool, \
             tc.tile_pool(name=f"hsb{r}", bufs=1) as hsb_pool, \
             tc.tile_pool(name=f"degps{r}", bufs=1, space="PSUM") as degps_pool, \
             tc.tile_pool(name=f"bcps{r}", bufs=1, space="PSUM") as bcps_pool, \
             tc.tile_pool(name=f"adjs1{r}", bufs=4) as adjs1_pool:
            xt_t = xw_pool.tile([P, FT, ROWS], BF16)
            nc.sync.dma_start(
                xt_t[:], xt_d.ap().rearrange("(f p) i -> p f i", p=P))
            w_t = xw_pool.tile([P, FT, C], BF16)
            nc.sync.dma_start(
                w_t[:], w_d.ap().rearrange("(f p) c -> p f c", p=P))

            h_sb = hsb_pool.tile([P, MT, C], BF16)
            for mt in range(MT):
                h_ps = hps_pool.tile([P, C], F32)
                for ft in range(FT):
                    nc.tensor.matmul(
                        h_ps[:],
                        lhsT=xt_t[:, ft, mt * P:(mt + 1) * P],
                        rhs=w_t[:, ft, :],
                        start=(ft == 0), stop=(ft == FT - 1))
                nc.scalar.copy(h_sb[:, mt, :], h_ps[:])
            nc.sync.dma_start(
                h_bounce.rearrange("(m p) c -> p m c", p=P), h_sb[:])

            if sim_mode:
                nc.sync.dma_start(h_full[0:ROWS, :], h_bounce[:])
            else:
                nc.gpsimd.collective_compute(
                    "AllGather", mybir.AluOpType.bypass,
                    replica_groups=[list(range(NCORES))],
                    ins=[h_bounce.opt()], outs=[h_full.opt()])

            # h_full -> SBUF in 4 chunks of 2MB (becomes g in phase 4)
            for q in range(4):
                nc.sync.dma_start(
                    hg_t[:, q * (KT // 4):(q + 1) * (KT // 4), :],
                    h_full[q * (N // 4):(q + 1) * (N // 4), :].rearrange(
                        "(k p) c -> p k c", p=P))

            # ------------ Phase 2: deg = row sums of A shard -------
            # fp8 DoubleRow packs: two j-rows per partition, ones weights
            deg_ps = [degps_pool.tile([1, C], F32, tag=f"degp{i}",
                                      name=f"degp{i}")
                      for i in range(2)]
            for dq in range(NDR // 2):      # DMA two DR packs at once
                pk8 = adjs1_pool.tile([P, 2, 2048], FP8, tag="pk8")
                nc.sync.dma_start(
                    pk8[:], adjt8_d.ap()[2 * dq:2 * dq + 2].rearrange(
                        "s p i -> p s i"))
                for s in range(2):
                    q = 2 * dq + s
                    r3 = pk8[:, s, :].rearrange(
                        "p (two i) -> p two i", two=2)
                    for half in range(2):
                        nc.tensor.matmul(
                            deg_ps[half][:],
                            lhsT=ones8_t[:, :, 0:1],
                            rhs=r3[:, :, half * C:(half + 1) * C],
                            perf_mode=mybir.MatmulPerfMode.DoubleRow,
                            start=(q == 0), stop=(q == NDR - 1))
            deg_sb = misc_pool.tile([1, ROWS], F32, tag="degsb")
            for half in range(2):
                nc.vector.tensor_copy(
                    deg_sb[:, half * C:(half + 1) * C], deg_ps[half][:])
            nc.sync.dma_start(deg_b[:], deg_sb[:])

            if sim_mode:
                nc.sync.dma_start(deg_all[0:1, :], deg_b[:])
            else:
                nc.gpsimd.collective_compute(
                    "AllGather", mybir.AluOpType.bypass,
                    replica_groups=[list(range(NCORES))],
                    ins=[deg_b.opt()], outs=[deg_all.opt()])

            # dinv for the core's own rows, broadcast across partitions
            # (no AllGather dependency; overlaps the deg AllGather)
            rrow = misc_pool.tile([1, ROWS], F32, tag="rrow")
            nc.vector.reciprocal(rrow[:], deg_sb[:])
            drow = misc_pool.tile([1, ROWS], F32, tag="drow")
            nc.scalar.sqrt(drow[:], rrow[:])
            bc_ps = bcps_pool.tile([P, ROWS], F32)
            for half in range(2):
                nc.tensor.matmul(
                    bc_ps[:, half * C:(half + 1) * C],
                    lhsT=ones1_t[:],
                    rhs=drow[:, half * C:(half + 1) * C],
                    start=True, stop=True)
            nc.vector.tensor_copy(dinvr_bc[:], bc_ps[:])

        # ---------------- Phase 3: dinv (all nodes) ---------------
        # natural [64, 128] load (contiguous rows), then PE transpose to
        # [128, 64] — avoids an 8192-descriptor 4-byte-gather DMA on the
        # critical path
        with tc.tile_pool(name=f"tps{r}", bufs=1, space="PSUM") as tps_pool:
            degkt_sb = misc_pool.tile([KT, P], F32, tag="degkt")
            nc.sync.dma_start(
                degkt_sb[:], deg_all.rearrange("c (m p) -> (c m) p", p=P))
            ident_t = misc_pool.tile([KT, KT], F32, tag="ident")
            make_identity(nc, ident_t[:])
            tp_ps = tps_pool.tile([P, KT], F32)
            nc.tensor.transpose(tp_ps[:], degkt_sb[:], ident_t[:])
            degk_sb = misc_pool.tile([P, KT], F32, tag="degk")
            nc.vector.tensor_copy(degk_sb[:], tp_ps[:])
        rink = misc_pool.tile([P, KT], F32, tag="rink")
        nc.vector.reciprocal(rink[:], degk_sb[:])
        dinv_sb = misc_pool.tile([P, KT], F32, tag="dinv")
        nc.scalar.sqrt(dinv_sb[:], rink[:])

        if parts == "pre":
            # just flush dinv so the program has an output write
            nc.sync.dma_start(out_d.ap()[0:P, 0:KT], dinv_sb[:])
            return

        _mm_tail(nc, tc, r, hg_t, dinv_sb, dinvr_bc, bias_pp, io, adjt_d)


def _mm_tail(nc, tc, r, hg_t, dinv_sb, dinvr_bc, bias_pp, io, adjt_d):
        out_d = io[5]
        # ---------------- Phase 4+5+6 -----------------------------
        with tc.tile_pool(name=f"adjs2{r}", bufs=5) as adjs2_pool, \
             tc.tile_pool(name=f"mmps{r}", bufs=1, space="PSUM") as mmps_pool, \
             tc.tile_pool(name=f"ep{r}", bufs=4) as ep_pool:
            # Phase 4: g = dinv * h, in place on the resident h tile
            for kt in range(KT):
                sl = hg_t[:, kt, :]
                nc.vector.tensor_scalar_mul(sl, sl, dinv_sb[:, kt:kt + 1])

            # out.T accumulators: one [128, 1024] (2 PSUM banks) per chunk
            mm_ps = [mmps_pool.tile([P, ROWS], F32, tag=f"mm{cc}",
                                    name=f"mm{cc}")
                     for cc in range(CC)]

            def mm(cc, kt, strip_ap, start, stop):
                for half in range(2):
                    nc.tensor.matmul(
                        mm_ps[cc][:, half * C:(half + 1) * C],
                        lhsT=hg_t[:, kt, cc * P:(cc + 1) * P],
                        rhs=strip_ap[:, half * C:(half + 1) * C],
                        start=start, stop=stop)

            def load_pack(pk):
                t = adjs2_pool.tile([P, SPK, ROWS], BF16, tag="pk2",
                                    name=f"pk2_{pk}")
                nc.sync.dma_start(
                    t[:], adjt_d.ap()[SPK * pk:SPK * (pk + 1)].rearrange(
                        "s p i -> p s i"))
                return t

            for pk in range(NPK - NTAILPK):
                t = load_pack(pk)
                for s in range(SPK):
                    kt = SPK * pk + s
                    for cc in range(CC):
                        mm(cc, kt, t[:, s, :],
                           start=(kt == 0), stop=False)

            # trailing packs: per-cc bursts so each chunk's epilogue
            # overlaps the next chunk's matmuls on the PE
            tail_tiles = [load_pack(pk)
                          for pk in range(NPK - NTAILPK, NPK)]
            ktail0 = SPK * (NPK - NTAILPK)
            for cc in range(CC):
                for kt in range(ktail0, KT):
                    ti = (kt - ktail0) // SPK
                    s = (kt - ktail0) % SPK
                    mm(cc, kt, tail_tiles[ti][:, s, :],
                       start=False, stop=(kt == KT - 1))
                t1 = ep_pool.tile([P, ROWS], F32, tag="t1")
                nc.vector.tensor_mul(t1[:], mm_ps[cc][:], dinvr_bc[:])
                t2 = ep_pool.tile([P, ROWS], F32, tag="t2")
                nc.scalar.activation(
                    t2[:], t1[:], mybir.ActivationFunctionType.Lrelu,
                    bias=bias_pp[:, cc:cc + 1], alpha=0.01)
                nc.sync.dma_start(out_d.ap()[cc * P:(cc + 1) * P, :], t2[:])


def build_kernel(reps: int = 1, sim_mode: bool = False, parts: str = "all"):
    """Build and compile the SPMD Bass program (identical on all 8 cores).

    reps > 1 repeats the whole pipeline inside one NEFF (timing only)."""
    nc = bacc.Bacc("TRN2", target_bir_lowering=False, debug=False,
                   num_devices=NCORES)

    adjt_d = nc.dram_tensor("adjt", [KT, P, ROWS], BF16, kind="ExternalInput")
    adjt8_d = nc.dram_tensor("adjt8", [NDR, P, 2048], FP8, kind="ExternalInput")
    xt_d = nc.dram_tensor("xt", [C, ROWS], BF16, kind="ExternalInput")
    w_d = nc.dram_tensor("w", [C, C], BF16, kind="ExternalInput")
    biasc_d = nc.dram_tensor("biasc", [C], F32, kind="ExternalInput")
    out_d = nc.dram_tensor("out", [C, ROWS], F32, kind="ExternalOutput")
    io = (adjt_d, adjt8_d, xt_d, w_d, biasc_d, out_d)

    with tile.TileContext(nc) as tc:
        with tc.tile_pool(name="dram", bufs=1, space="DRAM") as dram:
            if reps == 0:
                # near-empty program with the same I/O signature: used by
                # test.py to measure the dispatch floor
                with tc.tile_pool(name="nullp", bufs=1) as np_pool:
                    z = np_pool.tile([P, CC], F32)
                    nc.sync.dma_start(
                        z[:], biasc_d.ap().rearrange("(cc p) -> p cc", p=P))
            for r in range(reps):
                _emit(nc, tc, dram, io, r, sim_mode=sim_mode, parts=parts)

    nc.compile()
    return nc


def prepare_inputs(x, adj, weightr, weightd, bias):
    """Host-side sharding/layout. Returns in_maps for the 8 cores."""
    x = np.asarray(x, dtype=np.float32)
    adj = np.asarray(adj, dtype=np.float32)
    weightr = np.asarray(weightr, dtype=np.float32)
    weightd = np.asarray(weightd, dtype=np.float32)
    bias = np.ascontiguousarray(np.asarray(bias, dtype=np.float32))

    wr16 = weightr.astype(ml_dtypes.bfloat16)
    wd16 = weightd.astype(ml_dtypes.bfloat16)
    idx = np.arange(ROWS)
    # A values are only 0/1/2: build uint8 once, then LUT-cast (fast + exact)
    lut16 = np.array([0x0000, 0x3F80, 0x4000], dtype=np.uint16)  # bf16 bits
    lut8 = np.array([0x00, 0x38, 0x40], dtype=np.uint8)          # e4m3 bits

    in_maps = []
    for c in range(NCORES):
        rows = slice(c * ROWS, (c + 1) * ROWS)
        ai = adj[rows, :].T.astype(np.uint8)             # [N, ROWS] 0/1
        ai[c * ROWS + idx, idx] += 1                     # fold in self-loop
        adjt = lut16[ai].view(ml_dtypes.bfloat16).reshape(KT, P, ROWS)
        # DoubleRow packs: [32, 128, 2048], row p = [j=q*256+p | j=q*256+128+p]
        adjt8 = np.ascontiguousarray(
            lut8[ai].view(ml_dtypes.float8_e4m3)
            .reshape(NDR, 2, P, ROWS).transpose(0, 2, 1, 3)
        ).reshape(NDR, P, 2048)
        xt = np.ascontiguousarray(x[rows, :].T).astype(ml_dtypes.bfloat16)
        w = wr16 if c < NCORES // 2 else wd16
        in_maps.append({"adjt": adjt, "adjt8": adjt8, "xt": xt, "w": w,
                        "biasc": bias})
    return in_maps


_NC_CACHE = {}


def kernel(x, adj, weightr, weightd, bias):
    if "nc" not in _NC_CACHE:
        _NC_CACHE["nc"] = build_kernel(reps=1)
    nc = _NC_CACHE["nc"]
    in_maps = prepare_inputs(x, adj, weightr, weightd, bias)
    res = run_bass_kernel_spmd(nc, in_maps, list(range(NCORES)))
    out = np.concatenate(
        [np.ascontiguousarray(res.results[c]["out"].T) for c in range(NCORES)],
        axis=0)
    return out
